# revision 8
# baseline (speedup 1.0000x reference)
"""Trainium2 Bass kernel for nn_Norm_25795573580460.

Reference computation (B=8192 systems, L=2048 points each):
    xr = x.reshape(B, L, 3); mu = mean(xr, axis=1)
    sq = sum((xr-mu)^2, -1); std = sqrt(sum(softmax(sq)+EPS, 1)/L)
      -> softmax sums to 1 exactly, so std == sqrt((1+L*EPS)/L) (constant)
    init = proj^T @ (xr - mu) per batch  (3x3), row-normalized, Gram-Schmidt
    frame = gsp(init)^T; bias = frame @ b
    out[i] = g[i mod B] * (x[i] - mu[i div L]) / std + bias[i mod B]

Device work is split into two SPMD launches over 8 cores (batch-sharded,
1024 batches per core):
  phase 1: per-batch stats — 9 dot products sum_s(proj_k * x_j), column
           sums of x and proj (fused mul+reduce on VectorE via
           scalar_tensor_tensor accum, and ScalarE activation accum).
  (host)   tiny 3x3 per-batch math: init, Gram-Schmidt, frame, bias;
           exchange of the mod-B-indexed bias/g tables (the .repeat(l,1)
           tiling makes every core need all B bias rows).
  phase 2: out = gx_row * (x - mu) + bias_row, elementwise at DMA roofline.
"""

import json

import numpy as np

import concourse.bass as bass
import concourse.tile as tile
from concourse import mybir
from concourse.bass_utils import run_bass_kernel_spmd

# ---------------------------------------------------------------- constants
B = 8192          # systems
L = 2048          # points per system
EPS = 1e-5
NCORES = 8
BC = B // NCORES  # 1024 batches per core
P = 128           # SBUF partitions
NG = BC // P      # 8 batch-groups per core
F = L * 3         # 6144 floats per batch row
F32 = mybir.dt.float32

_ALU = mybir.AluOpType
_ACT = mybir.ActivationFunctionType

# ------------------------------------------------- walrus single-wait patch
# This container's walrus build rejects instructions carrying more than one
# sem-wait ("Too many sync wait commands"), while Tile freely attaches
# several.  Rewrite the BIR JSON at compile time: move extra waits onto
# fresh single-wait NoOps inserted before the instruction (same engine, so
# per-engine order — and therefore semantics — is preserved).
_FIX_N = [0]


def _split_multiwait_bir(bir_json):
    m = json.loads(bir_json.decode() if isinstance(bir_json, (bytes, bytearray)) else bir_json)
    changed = False
    for f in m.get("functions", []):
        for blk in f.get("blocks", []):
            out = []
            for ins in blk.get("instructions", []):
                si = ins.get("sync_info")
                if si:
                    w = si.get("on_wait") or []
                    if len(w) > 1:
                        for extra in w[:-1]:
                            _FIX_N[0] += 1
                            out.append({
                                "engine": ins["engine"],
                                "ins": [],
                                "outs": [],
                                "name": f"I-waitfix-{_FIX_N[0]}",
                                "opcode": "NoOp",
                                "sync_info": {"on_wait": [extra], "on_update": []},
                            })
                        si["on_wait"] = [w[-1]]
                        changed = True
                out.append(ins)
            blk["instructions"] = out
    return json.dumps(m).encode() if changed else bir_json


_PATCHED = [False]


def _apply_walrus_patch():
    if _PATCHED[0]:
        return
    import hashlib
    import shutil
    import tempfile
    from pathlib import Path

    from concourse import bass_utils as _bu
    from concourse import bass2jax as _b2j

    # Skip the BIR simulator during compile — it replays every DMA byte
    # against physical-size memory images (tens of MB here) and turns a
    # sub-second walrus run into tens of minutes.
    def _fast_verify_and_optimise(tmpdir, inp="bir.json", outp="file.neff",
                                  arch=None, *, dve_root=None):
        cmd = [
            _bu.get_walrus_driver(),
            "--pass",
            ",".join([
                "birverifier", "runtime_memory_reservation", "lower_act",
                "lower_dve", "lower_ap_offset", "codegen", "neff_packager",
            ]),
            "-i", inp,
            "--neff-output-filename", outp,
            "--enable-birsim=false",
            "--mem-mode=physical",
            "--policy=0",
            "--enable-ldw-opt=false",
            "--assign-static-dmas-to-sp=false",
            "--dram-page-size=256",
            "--enable-neff-debug-info=true",
            "--jobs", "8",
            *_bu.get_walrus_args(
                _bu.get_bir_arch(tmpdir, inp) if arch is None else arch,
                tmpdir, dve_root=dve_root,
            ),
        ]
        result = _bu.run_command(cmd, cwd=tmpdir)
        if result is not None:
            (Path(tmpdir) / "log.txt").write_text(result.stdout)
        return f"{tmpdir}/{outp}"

    _bu.bir_verify_and_optimise = _fast_verify_and_optimise

    orig = _bu.compile_bir_kernel
    cache_dir = Path(tempfile.gettempdir()) / "bass_neff_cache"
    cache_dir.mkdir(exist_ok=True)

    def patched(bir_json, tmpdir, neff_name="file.neff"):
        fixed = _split_multiwait_bir(bir_json)
        if isinstance(fixed, str):
            fixed = fixed.encode()
        key = hashlib.sha256(fixed + neff_name.encode()).hexdigest()[:24]
        hit = cache_dir / f"{key}_{neff_name}"
        target = Path(tmpdir) / "sg00" / neff_name
        if hit.exists():
            target.parent.mkdir(parents=True, exist_ok=True)
            shutil.copy(hit, target)
            return str(target)
        out = orig(fixed, tmpdir, neff_name)
        try:
            shutil.copy(out, hit)
        except OSError:
            pass
        return out

    _bu.compile_bir_kernel = patched
    _b2j.compile_bir_kernel = patched
    _PATCHED[0] = True


# ------------------------------------------------------------ phase 1 bass
def build_phase1(reps=None):
    """Per-core inputs xp/pp [NG, P, F]; outputs dots [NG, P, 12] (9 dot
    products sum_s p_k x_j at col 3k+j) and sums [NG, P, 8]
    (cols 0-2: sum_s x_j, cols 3-5: sum_s p_k)."""
    nc = bass.Bass()
    xp = nc.dram_tensor("xp", [NG, P, F], F32, kind="ExternalInput")
    pp = nc.dram_tensor("pp", [NG, P, F], F32, kind="ExternalInput")
    dots = nc.dram_tensor("dots", [NG, P, 9], F32, kind="ExternalOutput")
    sums = nc.dram_tensor("sums", [NG, P, 6], F32, kind="ExternalOutput")

    with tile.TileContext(nc) as tc:
        with (
            tc.tile_pool(name="xin", bufs=2) as xin,
            tc.tile_pool(name="pin", bufs=2) as pin,
            tc.tile_pool(name="stat", bufs=2) as stat,
            tc.tile_pool(name="scr", bufs=1) as scr,
        ):
            junk_d = scr.tile([P, L], F32, tag="junk_d")
            junk_a = scr.tile([P, L], F32, tag="junk_a")

            def body():
                for G in range(NG):
                    xt = xin.tile([P, F], F32, tag="xt")
                    pt = pin.tile([P, F], F32, tag="pt")
                    nc.sync.dma_start(xt[:], xp[G])
                    nc.sync.dma_start(pt[:], pp[G])
                    st_d = stat.tile([P, 9], F32, tag="st_d")
                    st_a = stat.tile([P, 6], F32, tag="st_a")
                    xr = xt.rearrange("p (s j) -> p s j", j=3)
                    pr = pt.rearrange("p (s j) -> p s j", j=3)
                    for k in range(3):
                        for j in range(3):
                            nc.vector.scalar_tensor_tensor(
                                out=junk_d[:],
                                in0=pr[:, :, k],
                                scalar=0.0,
                                in1=xr[:, :, j],
                                op0=_ALU.bypass,
                                op1=_ALU.mult,
                                accum_out=st_d[:, 3 * k + j : 3 * k + j + 1],
                            )
                    for j in range(3):
                        nc.scalar.activation(
                            out=junk_a[:],
                            in_=xr[:, :, j],
                            func=_ACT.Copy,
                            accum_out=st_a[:, j : j + 1],
                        )
                    for k in range(3):
                        nc.scalar.activation(
                            out=junk_a[:],
                            in_=pr[:, :, k],
                            func=_ACT.Copy,
                            accum_out=st_a[:, 3 + k : 3 + k + 1],
                        )
                    nc.sync.dma_start(dots[G], st_d[:])
                    nc.sync.dma_start(sums[G], st_a[:])

            if reps is None:
                body()
            else:
                with tc.For_i(0, reps, 1):
                    body()
    return nc


# ------------------------------------------------------------ phase 2 bass
def build_phase2(reps=None):
    """Per-core: out[G,p,3s+j] = gxf[p,3s+j] * (xp[G,p,3s+j] + negmu[p,3G+j])
    + brf[p,3s+j].  gxf/brf are the mod-B g*inv_std and bias tables expanded
    to the 128-partition tiling on the host (pattern repeats every 4
    partitions, identical for every group and core)."""
    nc = bass.Bass()
    xp = nc.dram_tensor("xp", [NG, P, F], F32, kind="ExternalInput")
    negmu = nc.dram_tensor("negmu", [P, NG, 3], F32, kind="ExternalInput")
    gxf = nc.dram_tensor("gxf", [P, F], F32, kind="ExternalInput")
    brf = nc.dram_tensor("brf", [P, F], F32, kind="ExternalInput")
    out = nc.dram_tensor("out", [NG, P, F], F32, kind="ExternalOutput")

    with tile.TileContext(nc) as tc:
        with (
            tc.tile_pool(name="const", bufs=1) as const,
            tc.tile_pool(name="xin", bufs=2) as xin,
            tc.tile_pool(name="oout", bufs=2) as oout,
        ):
            gxt = const.tile([P, F], F32, tag="gxt")
            brt = const.tile([P, F], F32, tag="brt")
            mt = const.tile([P, NG * 3], F32, tag="mt")
            nc.sync.dma_start(gxt[:], gxf[:])
            nc.sync.dma_start(brt[:], brf[:])
            nc.sync.dma_start(mt[:], negmu.rearrange("p g j -> p (g j)"))

            def body():
                for G in range(NG):
                    xt = xin.tile([P, F], F32, tag="xt")
                    nc.sync.dma_start(xt[:], xp[G])
                    xr = xt.rearrange("p (s j) -> p s j", j=3)
                    for j in range(3):
                        nc.scalar.activation(
                            out=xr[:, :, j],
                            in_=xr[:, :, j],
                            func=_ACT.Identity,
                            bias=mt[:, 3 * G + j : 3 * G + j + 1],
                        )
                    nc.vector.tensor_mul(xt[:], xt[:], gxt[:])
                    ot = oout.tile([P, F], F32, tag="ot")
                    nc.vector.tensor_add(ot[:], xt[:], brt[:])
                    nc.sync.dma_start(out[G], ot[:])

            if reps is None:
                body()
            else:
                with tc.For_i(0, reps, 1):
                    body()
    return nc


# ------------------------------------------------------------- host pieces
def _host_glue(dots, musum, colsum, g, b):
    """Per-batch 3x3 math: init -> row-normalize -> Gram-Schmidt -> frame ->
    bias = frame @ b.  All [B, ...] sized, f64 for stability."""
    dots = dots.astype(np.float64)
    mu = musum.astype(np.float64) / L                       # [B,3]
    init = dots - colsum.astype(np.float64)[:, :, None] * mu[:, None, :]
    init = init / np.linalg.norm(init, axis=2, keepdims=True)

    def proj_uv(u, w):
        return (
            np.sum(w * u, -1, keepdims=True)
            / (np.sum(u * u, -1, keepdims=True) + EPS)
        ) * u

    u0 = init[:, 0]
    u1 = init[:, 1] - proj_uv(u0, init[:, 1])
    u2 = init[:, 2] - proj_uv(u0, init[:, 2]) - proj_uv(u1, init[:, 2])
    uu = np.stack([u0, u1, u2], axis=1)                     # [B,3,3]
    uu = uu / (np.linalg.norm(uu, axis=-1, keepdims=True) + EPS)
    frame = uu.transpose(0, 2, 1)                           # [B,3,3]
    bias = np.einsum("bij,bj->bi", frame, b.astype(np.float64))  # [B,3]
    return mu, bias


_KCACHE = {}


def _get(name, builder, reps=None):
    key = (name, reps)
    if key not in _KCACHE:
        _KCACHE[key] = builder(reps)
    return _KCACHE[key]


def _timed_min(nc, in_maps, n_warm=1, n_meas=4):
    import time as _time

    for _ in range(n_warm):
        run_bass_kernel_spmd(nc, in_maps, core_ids=list(range(NCORES)))
    ts = []
    for _ in range(n_meas):
        t0 = _time.time()
        run_bass_kernel_spmd(nc, in_maps, core_ids=list(range(NCORES)))
        ts.append(_time.time() - t0)
    return min(ts)


def measure_hw_time(R=33):
    """Estimate summed device time of phase1+phase2 (ns) via the
    repeat-loop delta: (T(R reps) - T(1 rep)) / (R - 1) per phase."""
    _apply_walrus_patch()
    rng = np.random.default_rng(0)
    xa = rng.standard_normal((NG, P, F)).astype(np.float32)
    pa = rng.standard_normal((NG, P, F)).astype(np.float32)
    ma = rng.standard_normal((P, NG, 3)).astype(np.float32)
    ga = rng.standard_normal((P, F)).astype(np.float32)
    ba = rng.standard_normal((P, F)).astype(np.float32)

    in1 = [{"xp": xa, "pp": pa} for _ in range(NCORES)]
    in2 = [{"xp": xa, "negmu": ma, "gxf": ga, "brf": ba} for _ in range(NCORES)]

    total_ns = 0.0
    for name, builder, ins in (
        ("p1", build_phase1, in1),
        ("p2", build_phase2, in2),
    ):
        t1 = _timed_min(_get(name + "t1", builder, 1), ins)
        tR = _timed_min(_get(name + "tR", builder, R), ins)
        per = (tR - t1) / (R - 1)
        print(f"[{name}] T(1)={t1*1e3:.1f} ms  T({R})={tR*1e3:.1f} ms  "
              f"per-rep={per*1e6:.1f} us")
        total_ns += per * 1e9
    return total_ns


def kernel(x, g, b, proj):
    _apply_walrus_patch()

    x = np.ascontiguousarray(x, dtype=np.float32)
    proj = np.ascontiguousarray(proj, dtype=np.float32)
    g = np.asarray(g, dtype=np.float32)
    b = np.asarray(b, dtype=np.float32)

    xs = x.reshape(NCORES, NG, P, F)
    ps = proj.reshape(NCORES, NG, P, F)

    # ---- phase 1: per-batch reductions on device
    nc1 = _get("p1", build_phase1)
    in1 = [{"xp": xs[c], "pp": ps[c]} for c in range(NCORES)]
    res1 = run_bass_kernel_spmd(nc1, in1, core_ids=list(range(NCORES)))

    dots = np.stack([r["dots"] for r in res1.results])      # [NC, NG, P, 9]
    sums = np.stack([r["sums"] for r in res1.results])      # [NC, NG, P, 6]
    dots = dots.reshape(B, 3, 3)                            # [B,3,3] (k,j)
    sums = sums.reshape(B, 6)
    musum, colsum = sums[:, 0:3], sums[:, 3:6]

    # ---- host: tiny per-batch 3x3 math + mod-B table build
    mu, bias = _host_glue(dots, musum, colsum, g, b)

    inv_std = float(np.sqrt(L / (1.0 + L * EPS)))
    gx = g.astype(np.float64) * inv_std                     # [B]
    # out row i uses g[i mod B] / bias[i mod B]; within a 128-partition tile
    # the needed row index is 2048*(p mod 4) + s — identical for all groups
    # and cores, so one [P, F] table serves everything.
    gx_row = np.repeat(gx.reshape(4, L)[:, :, None], 3, axis=2).reshape(4, F)
    br_row = bias.reshape(4, L, 3).reshape(4, F)
    gxf = np.ascontiguousarray(np.tile(gx_row, (P // 4, 1)), dtype=np.float32)
    brf = np.ascontiguousarray(np.tile(br_row, (P // 4, 1)), dtype=np.float32)

    negmu = (-mu).astype(np.float32).reshape(NCORES, NG, P, 3)

    # ---- phase 2: elementwise output
    nc2 = _get("p2", build_phase2)
    in2 = [
        {
            "xp": xs[c],
            "negmu": np.ascontiguousarray(negmu[c].transpose(1, 0, 2)),
            "gxf": gxf,
            "brf": brf,
        }
        for c in range(NCORES)
    ]
    res2 = run_bass_kernel_spmd(nc2, in2, core_ids=list(range(NCORES)))

    out = np.concatenate(
        [r["out"].reshape(BC * L, 3) for r in res2.results], axis=0
    )
    return out
